# revision 1
# baseline (speedup 1.0000x reference)
"""Trainium2 Bass kernel for the DenseSNN problem (4-layer LIF spiking MLP).

Strategy
--------
Data-parallel over batch: B=128 is split into 8 shards of 16, one per
NeuronCore, with weights replicated (no collectives at all).

Per core the time recurrence is restructured layer-at-a-time: layer l's
input spikes for ALL timesteps are known once layer l-1's LIF scan
finishes, so each layer becomes ONE batched matmul over all (t, b) pairs
(M = T*Bs = 1024 rows -> full PE utilization) followed by a sequential
64-step elementwise LIF scan on the Vector engine, run on the negated
membrane m̃ = -mem/th (the -1/th is folded into weights/bias host-side):

    m̃(t)  = beta*m̃(t-1) + c̃(t) + spk(t-1)     (STT + TT)
    spk(t) = (m̃(t) < -1)                        (tensor_scalar is_lt)

so the reset term is just "add yesterday's spikes" — no extra scaling op.

All matmul operands are bf16 (1 cycle/row on the PE vs 4 for fp32);
accumulation stays fp32 in PSUM. Spikes are exactly representable in
bf16 (0.0/1.0). The per-neuron bias is fused into the PSUM->SBUF
evacuation on the Scalar engine (activation Identity with a bias AP).

Layers are processed in column chunks so the next chunk's / next layer's
matmuls overlap the previous chunk's LIF scan; the output layer uses a
narrow final chunk so only the last 8 timesteps of scan trail the last
matmul.

Layout (per core)
-----------------
Spikes/x are t-major 4D [128 partitions, t, kt, b] so each scan step's
spike write is one contiguous 512B-per-partition block (keeps Tile's
byte-range dependency tracking precise; a strided write's bounding box
would serialize every next-layer matmul behind the whole scan). The
matmul rhs is then a 3D AP [128, nt, 16] which the PE streams at full
rate. Weights are pre-transposed + blocked host-side to [p, mt, kt, f]
so each matmul lhsT tile is w[:, mt, kt, :] = [K=128, M=128] and a whole
mt-chunk is one contiguous DMA, re-streamed per column chunk and spread
across two DMA queues (one queue sustains only ~150 GB/s).

Measured on 8 axon-tunneled TRN2 cores: ~397 us HW exec, output exactly
matches the fp32 reference (all zeros for this problem's inputs — layer 3
never crosses threshold; verified margins are ~20 sigma of the bf16
noise).
"""

import os
import sys

import numpy as np
import ml_dtypes

if "/opt/trn_rl_repo" not in sys.path:
    sys.path.insert(0, "/opt/trn_rl_repo")

T, B, D_IN, D_H, D_OUT = 64, 128, 1024, 2048, 1000
NCORES = 8
BS = B // NCORES           # 16 batch rows per core
COLS = T * BS              # 1024 (t, b) columns
NTC = 2                    # column chunks per layer
CHW = COLS // NTC          # 512 columns per chunk (one PSUM bank)
TPC = T // NTC             # timesteps per chunk

BF16 = ml_dtypes.bfloat16

_COMPILED = {}


# --------------------------------------------------------------------------
# Program construction
# --------------------------------------------------------------------------

def _build(params, debug=False):
    from concourse import bacc, tile, mybir

    beta1, th1, beta2, th2, beta3, th3, beta_o, th_o = params
    f32 = mybir.dt.float32
    bf = mybir.dt.bfloat16
    Al = mybir.AluOpType
    AF = mybir.ActivationFunctionType

    nc = bacc.Bacc(
        "TRN2", target_bir_lowering=False, debug=False, num_devices=NCORES
    )

    xT_d = nc.dram_tensor("xT", [128, T, 8, BS], bf, kind="ExternalInput")
    w1_d = nc.dram_tensor("w1T", [128, 16, 8, 128], bf, kind="ExternalInput")
    w2_d = nc.dram_tensor("w2T", [128, 16, 16, 128], bf, kind="ExternalInput")
    w3_d = nc.dram_tensor("w3T", [128, 16, 16, 128], bf, kind="ExternalInput")
    wo_d = nc.dram_tensor("woT", [128, 8, 16, 128], bf, kind="ExternalInput")
    b1_d = nc.dram_tensor("b1v", [128, 16], f32, kind="ExternalInput")
    b2_d = nc.dram_tensor("b2v", [128, 16], f32, kind="ExternalInput")
    b3_d = nc.dram_tensor("b3v", [128, 16], f32, kind="ExternalInput")
    bo_d = nc.dram_tensor("bov", [128, 8], f32, kind="ExternalInput")
    out_d = nc.dram_tensor("acc_out", [128, 8, BS], f32, kind="ExternalOutput")
    if debug:
        dbg_d = nc.dram_tensor("dbg_s", [128, 3, 16], f32, kind="ExternalOutput")

    with tile.TileContext(nc) as tc:
        with (
            tc.tile_pool(name="const", bufs=1) as cpool,
            tc.tile_pool(name="wpool", bufs=6) as wpool,
            tc.tile_pool(name="curp", bufs=3) as curpool,
            tc.tile_pool(name="psp", bufs=6, space="PSUM") as pspool,
        ):
            # Spikes and x live t-major [128, t, kt, b]: each scan step's
            # spike write is one contiguous 512B-per-partition block, so
            # Tile's byte-range dependency tracking stays precise and
            # next-layer matmuls can start as soon as the columns they read
            # exist (bounding boxes of strided writes would otherwise
            # serialize every matmul behind the whole scan).
            xT = cpool.tile([128, T, 8, BS], bf, tag="xT")
            bt = {}
            for nm, d, mt in (
                ("b1", b1_d, 16), ("b2", b2_d, 16),
                ("b3", b3_d, 16), ("bo", bo_d, 8),
            ):
                bt[nm] = cpool.tile([128, mt], f32, tag=nm, name=nm)
                nc.gpsimd.dma_start(out=bt[nm][:], in_=d[:])
            xq = (nc.gpsimd, nc.sync, nc.scalar)
            for q in range(4):
                xq[q % 3].dma_start(
                    out=xT[:, q * 16:(q + 1) * 16], in_=xT_d[:, q * 16:(q + 1) * 16]
                )
            sA = cpool.tile([128, T, 16, BS], bf, tag="sA")
            sB = cpool.tile([128, T, 16, BS], bf, tag="sB")

            def gemm_chunk(c0, cw, w_d, btile, KT, MT, rhs):
                """One column chunk [c0, c0+cw) of a layer's matmul.

                Returns the SBUF cur tile [128, cw//BS, MT*BS] (t-major so
                the per-timestep scan slices are flat contiguous) in bf16
                with the bias already added.

                Weights/bias arrive pre-scaled by -1/th on the host, so the
                tile holds c̃ = -cur/th and the LIF scan runs on the negated
                membrane m̃ = -mem/th (threshold crossing = m̃ < -1).
                """
                nt = cw // BS
                curt = curpool.tile([128, nt, MT * BS], bf, tag="cur")
                for mt in range(MT):
                    wt = wpool.tile([128, KT, 128], bf, tag="wt")
                    # alternate DMA queues: one queue sustains ~150 GB/s and
                    # the weight restream needs more than that to stay ahead
                    weng = nc.sync if mt % 2 == 0 else nc.scalar
                    weng.dma_start(out=wt[:], in_=w_d[:, mt])
                    ps = pspool.tile([128, cw], f32, tag="ps")
                    for kt in range(KT):
                        nc.tensor.matmul(
                            ps[:],
                            wt[:, kt, :],
                            rhs(kt, c0, cw),
                            start=(kt == 0),
                            stop=(kt == KT - 1),
                        )
                    nc.scalar.activation(
                        curt[:, :, mt * BS:(mt + 1) * BS], ps[:], AF.Identity,
                        bias=btile[:, mt:mt + 1], scale=1.0,
                    )
                return curt

            def lif_step(mem2, mtmp, t, cur_sl, beta, spike_out, spike_prev):
                """One LIF timestep on the negated membrane m̃ = -mem/th.

                    m̃(t)  = beta*m̃(t-1) + c̃(t) + spk(t-1)
                    spk(t) = (m̃(t) < -1)

                Three DVE ops, none in-place (in-place costs ~+90ns/op):
                A (STT) writes the scratch tile mtmp, B (TT, has a 2x bf16
                uop unlike STT) adds the previous spikes into the ping-pong
                state tile, TS emits the spikes.
                """
                mprev, mcur = mem2[(t + 1) % 2], mem2[t % 2]
                if t == 0:
                    nc.vector.scalar_tensor_tensor(
                        mcur[:], mprev[:], float(beta), cur_sl, Al.mult, Al.add,
                    )
                else:
                    nc.vector.scalar_tensor_tensor(
                        mtmp[:], mprev[:], float(beta), cur_sl, Al.mult, Al.add,
                    )
                    nc.vector.tensor_tensor(
                        mcur[:], mtmp[:], spike_prev, Al.add,
                    )
                nc.vector.tensor_scalar(
                    spike_out, mcur[:], -1.0, None, Al.is_lt,
                )

            def hidden_layer(li, w_d, bname, KT, rhs, s_out, beta,
                             chunks=((0, 512), (512, 512))):
                MT = 16
                mem2 = (
                    cpool.tile([128, MT * BS], bf, tag="mem0", name=f"mem0_{li}"),
                    cpool.tile([128, MT * BS], bf, tag="mem1", name=f"mem1_{li}"),
                )
                mtmp = cpool.tile(
                    [128, MT * BS], bf, tag="mtmp", name=f"mtmp_{li}"
                )
                nc.vector.memset(mem2[1][:], 0.0)
                for c0, cw in chunks:
                    curt = gemm_chunk(c0, cw, w_d, bt[bname], KT, MT, rhs)
                    for ti in range(cw // BS):
                        t = c0 // BS + ti
                        lif_step(
                            mem2, mtmp, t, curt[:, ti], beta,
                            s_out[:, t],
                            s_out[:, t - 1] if t else None,
                        )

            def rhs_of(s):
                return lambda kt, c0, cw: s[:, c0 // BS:(c0 + cw) // BS, kt, :]

            # ---- layer 1: x (1024) -> 2048, spikes into sA
            hidden_layer(1, w1_d, "b1", 8, rhs_of(xT), sA, beta1)
            # ---- layer 2: sA -> 2048, spikes into sB
            hidden_layer(2, w2_d, "b2", 16, rhs_of(sA), sB, beta2)
            if debug:
                dbg = cpool.tile([128, 3, 16], f32, tag="dbg")
                nc.vector.tensor_reduce(
                    dbg[:, 0, :], sA[:].rearrange("p t h b -> p h t b"),
                    mybir.AxisListType.XY, Al.add,
                )
                nc.vector.tensor_reduce(
                    dbg[:, 1, :], sB[:].rearrange("p t h b -> p h t b"),
                    mybir.AxisListType.XY, Al.add,
                )
            # ---- layer 3: sB -> 2048, spikes into sA (reused)
            hidden_layer(3, w3_d, "b3", 16, rhs_of(sB), sA, beta3)
            if debug:
                nc.vector.tensor_reduce(
                    dbg[:, 2, :], sA[:].rearrange("p t h b -> p h t b"),
                    mybir.AxisListType.XY, Al.add,
                )
                nc.sync.dma_start(out=dbg_d[:], in_=dbg[:])

            # ---- output layer: sA -> 1024 (1000 padded), accumulate spikes.
            # The Lo scan trails the last matmul, so its step rate IS the
            # kernel tail. Small DVE ops are drain/sem-wait bound when every
            # op depends on its predecessor; split the neurons into two
            # independent half-chains (mt 0-3 / mt 4-7) and interleave their
            # ops so each op's producer is >=2 queue positions back and the
            # drains overlap the other chain's execution.
            MT = 8
            HW2 = MT * BS // 2     # 64 elems per half
            memo2 = []
            mtmpo = []
            spko = []
            for h in range(2):
                memo2.append((
                    cpool.tile([128, HW2], bf, tag=f"memo0{h}", name=f"memo0{h}"),
                    cpool.tile([128, HW2], bf, tag=f"memo1{h}", name=f"memo1{h}"),
                ))
                mtmpo.append(cpool.tile(
                    [128, HW2], bf, tag=f"mtmpo{h}", name=f"mtmpo{h}"
                ))
                spko.append([
                    cpool.tile([128, HW2], bf, tag=f"spko{h}_{i}",
                               name=f"spko{h}_{i}")
                    for i in range(8)
                ])
                nc.vector.memset(memo2[h][1][:], 0.0)
            acc = cpool.tile([128, MT * BS], f32, tag="acc")
            nc.gpsimd.memset(acc[:], 0.0)
            for c0, cw in ((0, 448), (448, 448), (896, 128)):
                curt = gemm_chunk(c0, cw, wo_d, bt["bo"], 16, MT, rhs_of(sA))
                for ti in range(cw // BS):
                    t = c0 // BS + ti
                    cur2 = (curt[:, ti, :HW2], curt[:, ti, HW2:])
                    for h in range(2):   # A steps, interleaved
                        mprev = memo2[h][(t + 1) % 2]
                        dst = memo2[h][t % 2] if t == 0 else mtmpo[h]
                        nc.vector.scalar_tensor_tensor(
                            dst[:], mprev[:], float(beta_o), cur2[h],
                            Al.mult, Al.add,
                        )
                    if t > 0:
                        for h in range(2):   # B steps
                            nc.vector.tensor_tensor(
                                memo2[h][t % 2][:], mtmpo[h][:],
                                spko[h][(t - 1) % 8][:], Al.add,
                            )
                    for h in range(2):   # spike steps
                        nc.vector.tensor_scalar(
                            spko[h][t % 8][:], memo2[h][t % 2][:],
                            -1.0, None, Al.is_lt,
                        )
                    for h in range(2):
                        # acc += spk on GpSimd; 8 spike slots give it slack
                        # so it never gates the DVE scan chain
                        nc.gpsimd.tensor_tensor(
                            acc[:, h * HW2:(h + 1) * HW2],
                            acc[:, h * HW2:(h + 1) * HW2],
                            spko[h][t % 8][:], Al.add,
                        )

            nc.sync.dma_start(out=out_d[:], in_=acc[:])

    nc.compile()
    return nc


def _get_compiled(params, debug=False):
    key = (params, debug)
    if key not in _COMPILED:
        _COMPILED[key] = _build(params, debug=debug)
    return _COMPILED[key]


# --------------------------------------------------------------------------
# Host-side data prep
# --------------------------------------------------------------------------

def _block_weights(w, KT, MT):
    """[M, K] fp32 -> [128, MT, KT, 128] bf16 with out[p, mt, kt, f] =
    w[mt*128 + f, kt*128 + p]."""
    M, K = w.shape
    assert M == MT * 128 and K == KT * 128
    return np.ascontiguousarray(
        w.reshape(MT, 128, KT, 128).transpose(3, 0, 2, 1)
    ).astype(BF16)


def _prep_inputs(inputs):
    x = np.asarray(inputs["x_seq"], np.float32)

    # The kernel runs the LIF scan on the negated membrane m̃ = -mem/th, so
    # every layer's weights/bias are pre-scaled by -1/th (exact sign flip
    # when th == 1).
    ths = {k: float(np.asarray(inputs[k], np.float32))
           for k in ("th1", "th2", "th3", "th_out")}
    for k, v in ths.items():
        assert v > 0, f"negated-membrane transform requires {k} > 0, got {v}"

    w1 = np.asarray(inputs["w1"], np.float32) * (-1.0 / ths["th1"])
    w2 = np.asarray(inputs["w2"], np.float32) * (-1.0 / ths["th2"])
    w3 = np.asarray(inputs["w3"], np.float32) * (-1.0 / ths["th3"])
    wo = np.asarray(inputs["wo"], np.float32) * (-1.0 / ths["th_out"])

    wo_p = np.zeros((1024, D_H), np.float32)
    wo_p[:D_OUT] = wo

    shared = {
        "w1T": _block_weights(w1, 8, 16),
        "w2T": _block_weights(w2, 16, 16),
        "w3T": _block_weights(w3, 16, 16),
        "woT": _block_weights(wo_p, 16, 8),
    }
    for nm, b, thk, mt in (
        ("b1v", inputs["b1"], "th1", 16),
        ("b2v", inputs["b2"], "th2", 16),
        ("b3v", inputs["b3"], "th3", 16),
    ):
        shared[nm] = np.ascontiguousarray(
            (np.asarray(b, np.float32) * (-1.0 / ths[thk])).reshape(mt, 128).T
        )
    bo_p = np.zeros(1024, np.float32)
    bo_p[:D_OUT] = np.asarray(inputs["bo"], np.float32) * (-1.0 / ths["th_out"])
    shared["bov"] = np.ascontiguousarray(bo_p.reshape(8, 128).T)

    # per-core x, t-major: [p, t, kt, b]
    xs = []
    xr = x.reshape(T, NCORES, BS, 8, 128)      # [t, c, b, kt, p]
    for c in range(NCORES):
        xc = xr[:, c].transpose(3, 0, 2, 1)    # [p, t, kt, b]
        xs.append(np.ascontiguousarray(xc).astype(BF16))
    return shared, xs


def _params_from_inputs(inputs):
    def f(v):
        return float(np.asarray(v, np.float32))
    return (
        float(np.clip(f(inputs["beta1"]), 0.0, 1.0)), f(inputs["th1"]),
        float(np.clip(f(inputs["beta2"]), 0.0, 1.0)), f(inputs["th2"]),
        float(np.clip(f(inputs["beta3"]), 0.0, 1.0)), f(inputs["th3"]),
        float(np.clip(f(inputs["beta_out"]), 0.0, 1.0)), f(inputs["th_out"]),
    )


def _assemble_output(results):
    out = np.zeros((B, D_OUT), np.float32)
    for c in range(NCORES):
        a = np.asarray(results[c]["acc_out"], np.float32)   # [128, 8, 16]
        out[c * BS:(c + 1) * BS] = (
            a.transpose(2, 1, 0).reshape(BS, 1024)[:, :D_OUT]
        )
    return out


# --------------------------------------------------------------------------
# Entry point
# --------------------------------------------------------------------------

def kernel(**inputs):
    from concourse.bass_utils import run_bass_kernel_spmd

    params = _params_from_inputs(inputs)
    debug = bool(int(os.environ.get("SNN_KERNEL_DEBUG", "0")))
    nc = _get_compiled(params, debug=debug)
    shared, xs = _prep_inputs(inputs)
    in_maps = [dict(shared, xT=xs[c]) for c in range(NCORES)]
    trace = bool(int(os.environ.get("SNN_KERNEL_TRACE", "0")))
    try:
        res = run_bass_kernel_spmd(
            nc, in_maps, list(range(NCORES)), trace=trace
        )
    except ModuleNotFoundError:
        res = run_bass_kernel_spmd(nc, in_maps, list(range(NCORES)))
    out = _assemble_output(res.results)
    kernel.last_results = res
    return out

